# revision 15
# baseline (speedup 1.0000x reference)
"""Trainium2 Bass kernel for nn_MixAttention (GAT-style mixed attention).

Strategy (8 cores, i-sharded over query rows, transposed compute):
  - Device computes scores in transposed layout [j on partitions, i free] so
    out^T += hc_chunk.T @ P^T_chunk contracts over partitions, no transposes.
  - All on-device working tiles fp16 (tolerance budget is ~2e-2; fp16 keeps
    score error ~5e-4): halves input DMA, unlocks DVE 2x (tensor_tensor) and
    4x (tensor_scalar) perf modes, and makes every matmul stationary fp16
    (1 cycle/row at any width).
  - Mask passed as complement-uint8 in a partition-major layout; fused as
    (-L*maskC + bcB) so exp underflows masked entries to exactly 0.
    L = 3e4 stays inside fp16 range (lrelu -> -300, exp -> ~1e-131 -> 0).
  - exp(alpha - M0) with a host-precomputed upper bound M0 (numerical shim
    only; cancels exactly in the rowsum division).
  - rowsum via M=1 ones-stationary matmul sharing the P^T moving operand.
  - Engine balance per chunk: ACT does PreluB + exp (bottleneck ~1.94us);
    DVE does tsA (4x) + lreluA STT + sum add (2x); GPSIMD does the mask STT
    and the hc PSUM->SBUF copies. Phase-0 grid math is batched per 8-chunk
    group so the tiny [128,1] ops amortize.
"""

import numpy as np

N = 8192
K = 256
F = 128
NC = 8
S = N // NC  # 1024 rows per core
NEG_L = 3.0e4
GRP = 4      # j-chunks per exp/matmul group

_BUILD_CACHE = {}


def _build_program(cA, cB):
    import contextlib

    import concourse.bacc as bacc
    import concourse.tile as tile
    from concourse import mybir

    nc = bacc.Bacc("TRN2", target_bir_lowering=False, debug=False, num_devices=NC)
    dt = mybir.dt
    AF = mybir.ActivationFunctionType
    OP = mybir.AluOpType
    f16 = dt.float16

    hctxT = nc.dram_tensor("hctxT", [K, N], f16, kind="ExternalInput")
    hstrT = nc.dram_tensor("hstrT", [K, N], f16, kind="ExternalInput")
    hctxT_my = nc.dram_tensor("hctxT_my", [K, S], f16, kind="ExternalInput")
    hstrT_my = nc.dram_tensor("hstrT_my", [K, S], f16, kind="ExternalInput")
    wvA = nc.dram_tensor("wvA", [K, F + 1], f16, kind="ExternalInput")
    vA = nc.dram_tensor("vA", [K, 2], f16, kind="ExternalInput")
    uB = nc.dram_tensor("uB", [K, 3], f16, kind="ExternalInput")
    maskP = nc.dram_tensor("maskP", [128, (N // 128) * S], dt.uint8,
                           kind="ExternalInput")
    negm0 = nc.dram_tensor("negm0", [128, 1], dt.float32, kind="ExternalInput")
    outT = nc.dram_tensor("outT", [F, S], dt.float32, kind="ExternalOutput")

    NCH = N // 128   # 64 j-chunks
    KC = K // 128    # 2 contraction chunks
    NSLAB = 16
    SLABC = NCH // NSLAB
    G0 = 8           # j-chunks per phase-0 stream group
    W0 = G0 * 128

    with tile.TileContext(nc) as tc:
        with contextlib.ExitStack() as ctx:
            vecs = ctx.enter_context(tc.tile_pool(name="vecs", bufs=1))
            hcpool = ctx.enter_context(tc.tile_pool(name="hc", bufs=1))
            stp = ctx.enter_context(tc.tile_pool(name="stream", bufs=2))
            work = ctx.enter_context(tc.tile_pool(name="work", bufs=3))
            grpp = ctx.enter_context(tc.tile_pool(name="grp", bufs=2))
            slabp = ctx.enter_context(tc.tile_pool(name="slabp", bufs=2))

            # ---- small inputs ----
            vA_sb = [vecs.tile([128, 2], f16, name=f"vA{k}") for k in range(KC)]
            uB_sb = [vecs.tile([128, 3], f16, name=f"uB{k}") for k in range(KC)]
            wvA_sb = [vecs.tile([128, F + 1], f16, name=f"wvA{k}")
                      for k in range(KC)]
            negm0_sb = vecs.tile([128, 1], dt.float32, name="negm0_sb")
            nc.sync.dma_start(negm0_sb[:], negm0.ap())
            my_str = [stp.tile([128, S], f16, name=f"mystr{k}", tag=f"hst{k}", bufs=3)
                      for k in range(KC)]
            my_ctx = [stp.tile([128, S], f16, name=f"myctx{k}", tag=f"hct{k}", bufs=3)
                      for k in range(KC)]
            for k in range(KC):
                ks = slice(128 * k, 128 * (k + 1))
                nc.sync.dma_start(vA_sb[k][:], vA.ap()[ks, :])
                nc.sync.dma_start(uB_sb[k][:], uB.ap()[ks, :])
                nc.sync.dma_start(wvA_sb[k][:], wvA.ap()[ks, :])
                nc.sync.dma_start(my_str[k][:], hstrT_my.ap()[ks, :])
                nc.sync.dma_start(my_ctx[k][:], hctxT_my.ap()[ks, :])
            for k in range(KC):
                nc.scalar.activation(my_str[k][:], my_str[k][:], AF.Exp)

            # ---- src rows for my i-slice ----
            sigrow = work.tile([1, S], dt.float32, name="sigrow", tag="u")
            srcBraw = work.tile([1, S], dt.float32, name="srcBraw", tag="tA")
            srcArow = work.tile([1, S], dt.float32, name="srcArow", tag="tB")
            with tc.tile_pool(name="psrow", bufs=1, space="PSUM") as psrow:
                psr0 = psrow.tile([1, S], dt.float32, name="psr0")
                psr1 = psrow.tile([1, S], dt.float32, name="psr1")
                psra = psrow.tile([1, S], dt.float32, name="psra")
                for k in range(KC):
                    st, sp = (k == 0), (k == KC - 1)
                    for h in range(S // 512):
                        hs_ = slice(512 * h, 512 * (h + 1))
                        nc.tensor.matmul(psr0[:, hs_], uB_sb[k][:, 0:1],
                                         my_str[k][:, hs_], start=st, stop=sp)
                        nc.tensor.matmul(psr1[:, hs_], uB_sb[k][:, 2:3],
                                         my_str[k][:, hs_], start=st, stop=sp)
                        nc.tensor.matmul(psra[:, hs_], vA_sb[k][:, 0:1],
                                         my_ctx[k][:, hs_], start=st, stop=sp)
                nc.vector.tensor_copy(sigrow[:], psr0[:])
                nc.vector.tensor_copy(srcBraw[:], psr1[:])
                if cA != 0.0:
                    nc.vector.tensor_scalar_add(srcArow[:], psra[:], cA)
                else:
                    nc.vector.tensor_copy(srcArow[:], psra[:])

            srecrow = work.tile([1, S], dt.float32, name="srecrow", tag="u")
            srcBrow = work.tile([1, S], dt.float32, name="srcBrow", tag="tA")
            nc.vector.reciprocal(srecrow[:], sigrow[:])
            nc.vector.tensor_tensor(srcBrow[:], srcBraw[:], srecrow[:], OP.mult)
            if cB != 0.0:
                nc.vector.tensor_scalar_add(srcBrow[:], srcBrow[:], cB)

            ones_row = vecs.tile([1, 128], dt.float32, name="ones_row")
            nc.vector.memset(ones_row[:], 1.0)
            ones_colf = vecs.tile([128, 1], dt.float32, name="ones_colf")
            nc.vector.memset(ones_colf[:], 1.0)
            ones_col = vecs.tile([128, 1], f16, name="ones_col")
            nc.vector.tensor_copy(ones_col[:], ones_colf[:])

            # fp16 broadcast tiles of the src rows
            bcA = vecs.tile([128, S], f16, name="bcA")
            bcB = vecs.tile([128, S], f16, name="bcB")
            with tc.tile_pool(name="ps0c", bufs=1, space="PSUM") as ps0c:
                psbc = ps0c.tile([128, S], dt.float32, name="psbc")
                psbc2 = ps0c.tile([128, S], dt.float32, name="psbc2")
                for h in range(S // 512):
                    hs_ = slice(512 * h, 512 * (h + 1))
                    nc.tensor.matmul(psbc[:, hs_], ones_row[:], srcArow[:, hs_],
                                     start=True, stop=True)
                    nc.tensor.matmul(psbc2[:, hs_], ones_row[:], srcBrow[:, hs_],
                                     start=True, stop=True)
                nc.vector.tensor_copy(bcA[:], psbc[:])
                nc.vector.tensor_copy(bcB[:], psbc2[:])

            # ---- interleaved stream (phase 0) + score/attend (phase 1) ----
            # Per-engine queues are FIFO in program order, so phase-0 and
            # phase-1 are emitted interleaved per 8-chunk stream group: ACT
            # crunches PreluB/exp of group g while group g+1's DMA lands.
            gridp = ctx.enter_context(tc.tile_pool(name="gridp", bufs=4))
            sgrids, bgrids, agrids = {}, {}, {}
            hc4_sb = [hcpool.tile([128, 4 * F], f16, name=f"hc{c}")
                      for c in range(NCH // 4)]
            hc_of = lambda c: hc4_sb[c // 4][:, (c % 4) * F:(c % 4 + 1) * F]
            def emit_phase1(t, slab):
                sgrp = grpp.tile([128, GRP * S], f16, name="sgrp")
                Pgrp = grpp.tile([128, GRP * S], f16, name="Pgrp")
                for cc in range(GRP):
                    c = t * SLABC + cc
                    o = cc * S
                    gidx, gcol = c // G0, c % G0
                    bgc = bgrids[gidx][:, gcol:gcol + 1]
                    agc = agrids[gidx][:, gcol:gcol + 1]
                    tB = work.tile([128, S], f16, name="tB")
                    nc.scalar.activation(tB[:], bcB[:], AF.Prelu,
                                         bias=bgc, scale=1.0, alpha=0.01)
                    tA = work.tile([128, S], f16, name="tA")
                    if c % 14 == 13:
                        nc.scalar.activation(tA[:], bcA[:], AF.Prelu,
                                             bias=agc, scale=1.0, alpha=0.01)
                    else:
                        sA = work.tile([128, S], f16, name="sAt")
                        nc.vector.tensor_scalar(
                            sA[:], bcA[:], agc, None, OP.add)
                        sA01 = work.tile([128, S], f16, name="sA01")
                        nc.vector.tensor_scalar(
                            sA01[:], bcA[:], agc, 0.01, OP.add, OP.mult)
                        nc.vector.tensor_tensor(tA[:], sA[:], sA01[:], OP.max)
                    nc.vector.tensor_tensor(
                        sgrp[:, o:o + S], tA[:], tB[:], OP.add)
                nc.scalar.activation(Pgrp[:], sgrp[:], AF.Exp,
                                     bias=negm0_sb[:], scale=1.0)
                for cc in range(GRP):
                    o = cc * S
                    c = t * SLABC + cc
                    eng = nc.vector if c % 13 == 12 else nc.gpsimd
                    eng.tensor_tensor(Pgrp[:, o:o + S], Pgrp[:, o:o + S],
                                      slab[:, o:o + S], OP.mult)
                st = (t == 0)
                sp = (t == NSLAB - 1)
                for cc in range(GRP):
                    c = t * SLABC + cc
                    for h in range(S // 512):
                        hs_ = slice(cc * S + 512 * h, cc * S + 512 * (h + 1))
                        ps_ = slice(512 * h, 512 * (h + 1))
                        nc.tensor.matmul(outT_ps[:, ps_], hc_of(c),
                                         Pgrp[:, hs_],
                                         start=st and cc == 0,
                                         stop=sp and cc == GRP - 1)
                        nc.tensor.matmul(rs_ps[:, ps_], ones_col[:],
                                         Pgrp[:, hs_],
                                         start=st and cc == 0,
                                         stop=sp and cc == GRP - 1)

            slab_ring = {}
            with contextlib.ExitStack() as pctx:
                ps0 = pctx.enter_context(
                    tc.tile_pool(name="ps0", bufs=1, space="PSUM"))
                ps1 = pctx.enter_context(
                    tc.tile_pool(name="ps1", bufs=1, space="PSUM"))
                outT_ps = ps1.tile([F, S], dt.float32, name="outT_ps")
                rs_ps = ps1.tile([1, S], dt.float32, name="rs_ps")

                for g in range(NCH // G0):
                    # --- phase 0: stream group g ---
                    gs = slice(W0 * g, W0 * (g + 1))
                    hst = [stp.tile([128, W0], f16, name=f"hstg{k}",
                                    tag=f"hst{k}", bufs=3) for k in range(KC)]
                    hct = [stp.tile([128, W0], f16, name=f"hctg{k}",
                                    tag=f"hct{k}", bufs=3) for k in range(KC)]
                    for k in range(KC):
                        ks = slice(128 * k, 128 * (k + 1))
                        nc.sync.dma_start(hst[k][:], hstrT.ap()[ks, gs])
                        nc.sync.dma_start(hct[k][:], hctxT.ap()[ks, gs])
                        nc.scalar.activation(hst[k][:], hst[k][:], AF.Exp)
                    for t in (2 * g, 2 * g + 1):
                        slab = slabp.tile([128, SLABC * S], dt.uint8,
                                          name="slab", bufs=6)
                        nc.sync.dma_start(
                            slab[:],
                            maskP.ap()[:, t * SLABC * S:(t + 1) * SLABC * S])
                        slab_ring[t] = slab
                    psbg = ps0.tile([128, 3 * G0], dt.float32, name="psbg")
                    psb = psbg[:, 0:2 * G0]
                    psag = psbg[:, 2 * G0:3 * G0]
                    for cp in range(G0 // 4):
                        psA = ps0.tile([128, 4 * F], dt.float32, name="psA")
                        for half in range(4):
                            cc = 4 * cp + half
                            c = G0 * g + cc
                            cs = slice(128 * cc, 128 * (cc + 1))
                            fs = slice(F * half, F * (half + 1))
                            for k in range(KC):
                                st, sp = (k == 0), (k == KC - 1)
                                nc.tensor.matmul(psb[:, 2 * cc:2 * cc + 2],
                                                 hst[k][:, cs],
                                                 uB_sb[k][:, 0:2],
                                                 start=st, stop=sp)
                                nc.tensor.matmul(psA[:, fs], hct[k][:, cs],
                                                 wvA_sb[k][:, 0:F],
                                                 start=st, stop=sp)
                                nc.tensor.matmul(psag[:, cc:cc + 1],
                                                 hct[k][:, cs],
                                                 wvA_sb[k][:, F:F + 1],
                                                 start=st, stop=sp)
                        nc.vector.tensor_copy(hc4_sb[(G0 * g) // 4 + cp][:],
                                              psA[:])
                    sg = gridp.tile([128, G0], dt.float32, name="sgrid")
                    bg = gridp.tile([128, G0], dt.float32, name="bgrid")
                    ag = gridp.tile([128, G0], dt.float32, name="agrid")
                    sgrids[g], bgrids[g], agrids[g] = sg, bg, ag
                    nc.vector.reciprocal(sg[:], psb[:, 0::2])
                    nc.vector.tensor_tensor(bg[:], psb[:, 1::2],
                                            sg[:], OP.mult)
                    if cA != 0.0:
                        nc.vector.tensor_scalar_add(ag[:], psag, cA)
                    else:
                        nc.vector.tensor_copy(ag[:], psag)
                    if cB != 0.0:
                        nc.vector.tensor_scalar_add(bg[:], bg[:], cB)

                    # --- phase 1 for the PREVIOUS group (1-group lookahead) ---
                    if g > 0:
                        for ti in range(2):
                            tprev = 2 * (g - 1) + ti
                            emit_phase1(tprev, slab_ring[tprev])

                for ti in range(2):
                    tlast = NCH // G0 * 2 - 2 + ti
                    emit_phase1(tlast, slab_ring[tlast])

                # ---- normalize and write out ----
                rs_sb = work.tile([1, S], dt.float32, name="rs_sb", tag="tB")
                nc.vector.tensor_scalar_add(rs_sb[:], rs_ps[:], 1e-30)
                rrec = work.tile([1, S], dt.float32, name="rrec", tag="sAt")
                nc.vector.reciprocal_approx_fast(rrec[:], rs_sb[:])
                rbc_ps = ps0.tile([128, S], dt.float32, name="rbc_ps")
                for h in range(S // 512):
                    hs_ = slice(512 * h, 512 * (h + 1))
                    nc.tensor.matmul(rbc_ps[:, hs_], ones_row[:],
                                     rrec[:, hs_], start=True, stop=True)
                rbc = work.tile([128, S], dt.float32, name="rbcs", tag="u")
                nc.vector.tensor_copy(rbc[:], rbc_ps[:])
                out_sb = work.tile([F, S], dt.float32, name="out_sb", tag="tA")
                nc.vector.tensor_tensor(out_sb[:], outT_ps[:], rbc[:],
                                        OP.mult)
                nc.sync.dma_start(outT.ap(), out_sb[:])

    nc.compile()
    return nc


def kernel(h_context, h_structure, edge_index, Wc_w, Wc_b, Ws_w, Ws_b,
           ac_w, as_w, Ws_coff, Wc_coff):
    from concourse.bass_utils import run_bass_kernel_spmd

    h_context = np.asarray(h_context, np.float32)
    h_structure = np.asarray(h_structure, np.float32)
    Wc_w = np.asarray(Wc_w, np.float32)
    Wc_b = np.asarray(Wc_b, np.float32)
    Ws_w = np.asarray(Ws_w, np.float32)
    Ws_b = np.asarray(Ws_b, np.float32)
    ac_w = np.asarray(ac_w, np.float32)
    as_w = np.asarray(as_w, np.float32)
    ei = np.asarray(edge_index)

    wA = float(abs(np.float32(np.asarray(Ws_coff)[0, 0])))  # scales alpha_c
    wB = float(abs(np.float32(np.asarray(Wc_coff)[0, 0])))  # scales alpha_s

    vA_np = np.stack([Wc_w.T @ ac_w[0, :F], Wc_w.T @ ac_w[0, F:]], axis=1) * wA
    uB_np = np.stack([
        np.ones(K, np.float32),
        wB * (Ws_w.T @ as_w[0, F:]),   # dstB proj
        wB * (Ws_w.T @ as_w[0, :F]),   # srcB proj
    ], axis=1).astype(np.float32)
    cA = wA * float(Wc_b @ ac_w[0, :F] + Wc_b @ ac_w[0, F:])
    cB = wB * float(Ws_b @ as_w[0, :F] + Ws_b @ as_w[0, F:])

    key = (round(cA, 12), round(cB, 12))
    if key not in _BUILD_CACHE:
        _BUILD_CACHE[key] = _build_program(cA, cB)
    nc = _BUILD_CACHE[key]

    # edge-indicator adjacency, transposed + partition-major re-layout
    maskCT = np.zeros((N, N), np.uint8)
    maskCT[ei[1], ei[0]] = 1

    hctxT = np.ascontiguousarray(h_context.T.astype(np.float16))
    hstrT = np.ascontiguousarray(h_structure.T.astype(np.float16))
    vA_f16 = np.ascontiguousarray(vA_np.astype(np.float16))
    wvA_f16 = np.ascontiguousarray(
        np.concatenate([Wc_w.T, vA_np[:, 1:2]], axis=1).astype(np.float16))
    uB_f16 = np.ascontiguousarray(uB_np.astype(np.float16))

    # host M0 shim: upper bound of alpha per core (cancels in division)
    lrelu = lambda x: np.where(x > 0, x, 0.01 * x)
    srcA = h_context @ (vA_np[:, 0]) + cA          # wA folded
    dstA = h_context @ (vA_np[:, 1])
    e_str = np.exp(h_structure - h_structure.max(axis=1, keepdims=True))
    sm = e_str / e_str.sum(axis=1, keepdims=True)
    srcB = sm @ uB_np[:, 2] + cB
    dstB = sm @ uB_np[:, 1]
    dstA_max = float(dstA.max())
    dstB_max = float(dstB.max())

    in_maps = []
    for d in range(NC):
        sl = slice(S * d, S * (d + 1))
        m0_d = (lrelu(float(srcA[sl].max()) + dstA_max)
                + lrelu(float(srcB[sl].max()) + dstB_max))
        maskP = np.ascontiguousarray(
            maskCT[:, sl].reshape(N // 128, 128, S)
            .transpose(1, 0, 2).reshape(128, (N // 128) * S))
        in_maps.append({
            "hctxT": hctxT,
            "hstrT": hstrT,
            "hctxT_my": np.ascontiguousarray(hctxT[:, sl]),
            "hstrT_my": np.ascontiguousarray(hstrT[:, sl]),
            "wvA": wvA_f16,
            "vA": vA_f16,
            "uB": uB_f16,
            "maskP": maskP,
            "negm0": np.full((128, 1), -np.float32(m0_d), np.float32),
        })

    res = run_bass_kernel_spmd(nc, in_maps, core_ids=list(range(NC)))
    out = np.empty((N, F), np.float32)
    for d in range(NC):
        out[S * d:S * (d + 1), :] = res.results[d]["outT"].T

    # rows with no edges: reference gives uniform attention = mean of hc
    row_deg = np.zeros(N, np.int64)
    np.add.at(row_deg, ei[0], 1)
    empty = row_deg == 0
    if empty.any():
        hc_host = h_context @ Wc_w.T + Wc_b
        out[empty, :] = hc_host.mean(axis=0)

    return out


# revision 16
# speedup vs baseline: 1.0535x; 1.0535x over previous
"""Trainium2 Bass kernel for nn_MixAttention (GAT-style mixed attention).

Strategy (8 cores, i-sharded over query rows, transposed compute):
  - Device computes scores in transposed layout [j on partitions, i free] so
    out^T += hc_chunk.T @ P^T_chunk contracts over partitions, no transposes.
  - All on-device working tiles fp16 (tolerance budget is ~2e-2; fp16 keeps
    score error ~5e-4): halves input DMA, unlocks DVE 2x (tensor_tensor) and
    4x (tensor_scalar) perf modes, and makes every matmul stationary fp16
    (1 cycle/row at any width).
  - Mask passed as complement-uint8 in a partition-major layout; fused as
    (-L*maskC + bcB) so exp underflows masked entries to exactly 0.
    L = 3e4 stays inside fp16 range (lrelu -> -300, exp -> ~1e-131 -> 0).
  - exp(alpha - M0) with a host-precomputed upper bound M0 (numerical shim
    only; cancels exactly in the rowsum division).
  - rowsum via M=1 ones-stationary matmul sharing the P^T moving operand.
  - Engine balance per chunk: ACT does PreluB + exp (bottleneck ~1.94us);
    DVE does tsA (4x) + lreluA STT + sum add (2x); GPSIMD does the mask STT
    and the hc PSUM->SBUF copies. Phase-0 grid math is batched per 8-chunk
    group so the tiny [128,1] ops amortize.
"""

import numpy as np

N = 8192
K = 256
F = 128
NC = 8
S = N // NC  # 1024 rows per core
NEG_L = 3.0e4
GRP = 4      # j-chunks per exp/matmul group

_BUILD_CACHE = {}


def _build_program(cA, cB):
    import contextlib

    import concourse.bacc as bacc
    import concourse.tile as tile
    from concourse import mybir

    nc = bacc.Bacc("TRN2", target_bir_lowering=False, debug=False, num_devices=NC)
    dt = mybir.dt
    AF = mybir.ActivationFunctionType
    OP = mybir.AluOpType
    f16 = dt.float16

    hctxT = nc.dram_tensor("hctxT", [K, N], f16, kind="ExternalInput")
    hstrT = nc.dram_tensor("hstrT", [K, N], f16, kind="ExternalInput")
    hctxT_my = nc.dram_tensor("hctxT_my", [K, S], f16, kind="ExternalInput")
    hstrT_my = nc.dram_tensor("hstrT_my", [K, S], f16, kind="ExternalInput")
    wvA = nc.dram_tensor("wvA", [K, F + 1], f16, kind="ExternalInput")
    vA = nc.dram_tensor("vA", [K, 2], f16, kind="ExternalInput")
    uB = nc.dram_tensor("uB", [K, 3], f16, kind="ExternalInput")
    maskP = nc.dram_tensor("maskP", [128, (N // 128) * S], dt.uint8,
                           kind="ExternalInput")
    negm0 = nc.dram_tensor("negm0", [128, 1], dt.float32, kind="ExternalInput")
    outT = nc.dram_tensor("outT", [F, S], dt.float32, kind="ExternalOutput")

    NCH = N // 128   # 64 j-chunks
    KC = K // 128    # 2 contraction chunks
    NSLAB = 16
    SLABC = NCH // NSLAB
    G0 = 8           # j-chunks per phase-0 stream group
    W0 = G0 * 128

    with tile.TileContext(nc) as tc:
        with contextlib.ExitStack() as ctx:
            vecs = ctx.enter_context(tc.tile_pool(name="vecs", bufs=1))
            hcpool = ctx.enter_context(tc.tile_pool(name="hc", bufs=1))
            stp = ctx.enter_context(tc.tile_pool(name="stream", bufs=2))
            work = ctx.enter_context(tc.tile_pool(name="work", bufs=3))
            grpp = ctx.enter_context(tc.tile_pool(name="grp", bufs=2))
            slabp = ctx.enter_context(tc.tile_pool(name="slabp", bufs=2))

            # ---- small inputs ----
            vA_sb = [vecs.tile([128, 2], f16, name=f"vA{k}") for k in range(KC)]
            uB_sb = [vecs.tile([128, 3], f16, name=f"uB{k}") for k in range(KC)]
            wvA_sb = [vecs.tile([128, F + 1], f16, name=f"wvA{k}")
                      for k in range(KC)]
            negm0_sb = vecs.tile([128, 1], dt.float32, name="negm0_sb")
            nc.sync.dma_start(negm0_sb[:], negm0.ap())
            my_str = [stp.tile([128, S], f16, name=f"mystr{k}", tag=f"hst{k}", bufs=3)
                      for k in range(KC)]
            my_ctx = [stp.tile([128, S], f16, name=f"myctx{k}", tag=f"hct{k}", bufs=3)
                      for k in range(KC)]
            for k in range(KC):
                ks = slice(128 * k, 128 * (k + 1))
                nc.sync.dma_start(vA_sb[k][:], vA.ap()[ks, :])
                nc.sync.dma_start(uB_sb[k][:], uB.ap()[ks, :])
                nc.sync.dma_start(wvA_sb[k][:], wvA.ap()[ks, :])
                nc.sync.dma_start(my_str[k][:], hstrT_my.ap()[ks, :])
                nc.sync.dma_start(my_ctx[k][:], hctxT_my.ap()[ks, :])
            for k in range(KC):
                nc.scalar.activation(my_str[k][:], my_str[k][:], AF.Exp)

            # ---- src rows for my i-slice ----
            sigrow = work.tile([1, S], dt.float32, name="sigrow", tag="u")
            srcBraw = work.tile([1, S], dt.float32, name="srcBraw", tag="tA")
            srcArow = work.tile([1, S], dt.float32, name="srcArow", tag="tB")
            with tc.tile_pool(name="psrow", bufs=1, space="PSUM") as psrow:
                psr0 = psrow.tile([1, S], dt.float32, name="psr0")
                psr1 = psrow.tile([1, S], dt.float32, name="psr1")
                psra = psrow.tile([1, S], dt.float32, name="psra")
                for k in range(KC):
                    st, sp = (k == 0), (k == KC - 1)
                    for h in range(S // 512):
                        hs_ = slice(512 * h, 512 * (h + 1))
                        nc.tensor.matmul(psr0[:, hs_], uB_sb[k][:, 0:1],
                                         my_str[k][:, hs_], start=st, stop=sp)
                        nc.tensor.matmul(psr1[:, hs_], uB_sb[k][:, 2:3],
                                         my_str[k][:, hs_], start=st, stop=sp)
                        nc.tensor.matmul(psra[:, hs_], vA_sb[k][:, 0:1],
                                         my_ctx[k][:, hs_], start=st, stop=sp)
                nc.vector.tensor_copy(sigrow[:], psr0[:])
                nc.vector.tensor_copy(srcBraw[:], psr1[:])
                if cA != 0.0:
                    nc.vector.tensor_scalar_add(srcArow[:], psra[:], cA)
                else:
                    nc.vector.tensor_copy(srcArow[:], psra[:])

            srecrow = work.tile([1, S], dt.float32, name="srecrow", tag="u")
            srcBrow = work.tile([1, S], dt.float32, name="srcBrow", tag="tA")
            nc.vector.reciprocal(srecrow[:], sigrow[:])
            nc.vector.tensor_tensor(srcBrow[:], srcBraw[:], srecrow[:], OP.mult)
            if cB != 0.0:
                nc.vector.tensor_scalar_add(srcBrow[:], srcBrow[:], cB)

            ones_row = vecs.tile([1, 128], dt.float32, name="ones_row")
            nc.vector.memset(ones_row[:], 1.0)
            ones_colf = vecs.tile([128, 1], dt.float32, name="ones_colf")
            nc.vector.memset(ones_colf[:], 1.0)
            ones_col = vecs.tile([128, 1], f16, name="ones_col")
            nc.vector.tensor_copy(ones_col[:], ones_colf[:])

            # fp16 broadcast tiles of the src rows
            bcA = vecs.tile([128, S], f16, name="bcA")
            bcB = vecs.tile([128, S], f16, name="bcB")
            with tc.tile_pool(name="ps0c", bufs=1, space="PSUM") as ps0c:
                psbc = ps0c.tile([128, S], dt.float32, name="psbc")
                psbc2 = ps0c.tile([128, S], dt.float32, name="psbc2")
                for h in range(S // 512):
                    hs_ = slice(512 * h, 512 * (h + 1))
                    nc.tensor.matmul(psbc[:, hs_], ones_row[:], srcArow[:, hs_],
                                     start=True, stop=True)
                    nc.tensor.matmul(psbc2[:, hs_], ones_row[:], srcBrow[:, hs_],
                                     start=True, stop=True)
                nc.vector.tensor_copy(bcA[:], psbc[:])
                nc.vector.tensor_copy(bcB[:], psbc2[:])

            # ---- interleaved stream (phase 0) + score/attend (phase 1) ----
            # Per-engine queues are FIFO in program order, so phase-0 and
            # phase-1 are emitted interleaved per 8-chunk stream group: ACT
            # crunches PreluB/exp of group g while group g+1's DMA lands.
            gridp = ctx.enter_context(tc.tile_pool(name="gridp", bufs=4))
            sgrids, bgrids, agrids = {}, {}, {}
            hc4_sb = [hcpool.tile([128, 4 * F], f16, name=f"hc{c}")
                      for c in range(NCH // 4)]
            hc_of = lambda c: hc4_sb[c // 4][:, (c % 4) * F:(c % 4 + 1) * F]
            EGRP = 2  # chunks per exp group
            pend = []

            def emit_scores(c0, slab):
                sgrp = grpp.tile([128, EGRP * S], f16, name="sgrp", bufs=3)
                for cc in range(EGRP):
                    c = c0 + cc
                    o = cc * S
                    gidx, gcol = c // G0, c % G0
                    bgc = bgrids[gidx][:, gcol:gcol + 1]
                    agc = agrids[gidx][:, gcol:gcol + 1]
                    tB = work.tile([128, S], f16, name="tB")
                    nc.scalar.activation(tB[:], bcB[:], AF.Prelu,
                                         bias=bgc, scale=1.0, alpha=0.01)
                    tA = work.tile([128, S], f16, name="tA")
                    if c % 14 == 13:
                        nc.scalar.activation(tA[:], bcA[:], AF.Prelu,
                                             bias=agc, scale=1.0, alpha=0.01)
                    else:
                        sA = work.tile([128, S], f16, name="sAt")
                        nc.vector.tensor_scalar(
                            sA[:], bcA[:], agc, None, OP.add)
                        sA01 = work.tile([128, S], f16, name="sA01")
                        nc.vector.tensor_scalar(
                            sA01[:], bcA[:], agc, 0.01, OP.add, OP.mult)
                        nc.vector.tensor_tensor(tA[:], sA[:], sA01[:], OP.max)
                    nc.vector.tensor_tensor(
                        sgrp[:, o:o + S], tA[:], tB[:], OP.add)
                pend.append((c0, sgrp, slab))

            def emit_attend():
                c0, sgrp, slab = pend.pop(0)
                Pgrp = grpp.tile([128, EGRP * S], f16, name="Pgrp", bufs=2)
                nc.scalar.activation(Pgrp[:], sgrp[:], AF.Exp,
                                     bias=negm0_sb[:], scale=1.0)
                so = (c0 % SLABC) * S
                for cc in range(EGRP):
                    o = cc * S
                    c = c0 + cc
                    eng = nc.vector if c % 13 == 12 else nc.gpsimd
                    eng.tensor_tensor(Pgrp[:, o:o + S], Pgrp[:, o:o + S],
                                      slab[:, so + o:so + o + S], OP.mult)
                st = (c0 == 0)
                sp = (c0 == NCH - EGRP)
                for cc in range(EGRP):
                    c = c0 + cc
                    for h in range(S // 512):
                        hs_ = slice(cc * S + 512 * h, cc * S + 512 * (h + 1))
                        ps_ = slice(512 * h, 512 * (h + 1))
                        nc.tensor.matmul(outT_ps[:, ps_], hc_of(c),
                                         Pgrp[:, hs_],
                                         start=st and cc == 0,
                                         stop=sp and cc == EGRP - 1)
                        nc.tensor.matmul(rs_ps[:, ps_], ones_col[:],
                                         Pgrp[:, hs_],
                                         start=st and cc == 0,
                                         stop=sp and cc == EGRP - 1)

            def emit_phase1(t, slab):
                for gg in range(SLABC // EGRP):
                    emit_scores(t * SLABC + gg * EGRP, slab)
                    if len(pend) > 1:
                        emit_attend()

            slab_ring = {}
            slab_ring = {}
            with contextlib.ExitStack() as pctx:
                ps0 = pctx.enter_context(
                    tc.tile_pool(name="ps0", bufs=1, space="PSUM"))
                ps1 = pctx.enter_context(
                    tc.tile_pool(name="ps1", bufs=1, space="PSUM"))
                outT_ps = ps1.tile([F, S], dt.float32, name="outT_ps")
                rs_ps = ps1.tile([1, S], dt.float32, name="rs_ps")

                for g in range(NCH // G0):
                    # --- phase 0: stream group g ---
                    gs = slice(W0 * g, W0 * (g + 1))
                    hst = [stp.tile([128, W0], f16, name=f"hstg{k}",
                                    tag=f"hst{k}", bufs=3) for k in range(KC)]
                    hct = [stp.tile([128, W0], f16, name=f"hctg{k}",
                                    tag=f"hct{k}", bufs=3) for k in range(KC)]
                    for k in range(KC):
                        ks = slice(128 * k, 128 * (k + 1))
                        nc.sync.dma_start(hst[k][:], hstrT.ap()[ks, gs])
                        nc.sync.dma_start(hct[k][:], hctxT.ap()[ks, gs])
                        nc.scalar.activation(hst[k][:], hst[k][:], AF.Exp)
                    for t in (2 * g, 2 * g + 1):
                        slab = slabp.tile([128, SLABC * S], dt.uint8,
                                          name="slab", bufs=6)
                        nc.sync.dma_start(
                            slab[:],
                            maskP.ap()[:, t * SLABC * S:(t + 1) * SLABC * S])
                        slab_ring[t] = slab
                    psbg = ps0.tile([128, 3 * G0], dt.float32, name="psbg")
                    psb = psbg[:, 0:2 * G0]
                    psag = psbg[:, 2 * G0:3 * G0]
                    for cp in range(G0 // 4):
                        psA = ps0.tile([128, 4 * F], dt.float32, name="psA")
                        for half in range(4):
                            cc = 4 * cp + half
                            c = G0 * g + cc
                            cs = slice(128 * cc, 128 * (cc + 1))
                            fs = slice(F * half, F * (half + 1))
                            for k in range(KC):
                                st, sp = (k == 0), (k == KC - 1)
                                nc.tensor.matmul(psb[:, 2 * cc:2 * cc + 2],
                                                 hst[k][:, cs],
                                                 uB_sb[k][:, 0:2],
                                                 start=st, stop=sp)
                                nc.tensor.matmul(psA[:, fs], hct[k][:, cs],
                                                 wvA_sb[k][:, 0:F],
                                                 start=st, stop=sp)
                                nc.tensor.matmul(psag[:, cc:cc + 1],
                                                 hct[k][:, cs],
                                                 wvA_sb[k][:, F:F + 1],
                                                 start=st, stop=sp)
                        nc.vector.tensor_copy(hc4_sb[(G0 * g) // 4 + cp][:],
                                              psA[:])
                    sg = gridp.tile([128, G0], dt.float32, name="sgrid")
                    bg = gridp.tile([128, G0], dt.float32, name="bgrid")
                    ag = gridp.tile([128, G0], dt.float32, name="agrid")
                    sgrids[g], bgrids[g], agrids[g] = sg, bg, ag
                    nc.vector.reciprocal(sg[:], psb[:, 0::2])
                    nc.vector.tensor_tensor(bg[:], psb[:, 1::2],
                                            sg[:], OP.mult)
                    if cA != 0.0:
                        nc.vector.tensor_scalar_add(ag[:], psag, cA)
                    else:
                        nc.vector.tensor_copy(ag[:], psag)
                    if cB != 0.0:
                        nc.vector.tensor_scalar_add(bg[:], bg[:], cB)

                    # --- phase 1 for the PREVIOUS group (1-group lookahead) ---
                    if g > 0:
                        for ti in range(2):
                            tprev = 2 * (g - 1) + ti
                            emit_phase1(tprev, slab_ring[tprev])

                for ti in range(2):
                    tlast = NCH // G0 * 2 - 2 + ti
                    emit_phase1(tlast, slab_ring[tlast])
                while pend:
                    emit_attend()

                # ---- normalize and write out ----
                rs_sb = work.tile([1, S], dt.float32, name="rs_sb", tag="tB")
                nc.vector.tensor_scalar_add(rs_sb[:], rs_ps[:], 1e-30)
                rrec = work.tile([1, S], dt.float32, name="rrec", tag="sAt")
                nc.vector.reciprocal_approx_fast(rrec[:], rs_sb[:])
                rbc_ps = ps0.tile([128, S], dt.float32, name="rbc_ps")
                for h in range(S // 512):
                    hs_ = slice(512 * h, 512 * (h + 1))
                    nc.tensor.matmul(rbc_ps[:, hs_], ones_row[:],
                                     rrec[:, hs_], start=True, stop=True)
                rbc = work.tile([128, S], dt.float32, name="rbcs", tag="u")
                nc.vector.tensor_copy(rbc[:], rbc_ps[:])
                out_sb = work.tile([F, S], dt.float32, name="out_sb", tag="tA")
                nc.vector.tensor_tensor(out_sb[:], outT_ps[:], rbc[:],
                                        OP.mult)
                nc.sync.dma_start(outT.ap(), out_sb[:])

    nc.compile()
    return nc


def kernel(h_context, h_structure, edge_index, Wc_w, Wc_b, Ws_w, Ws_b,
           ac_w, as_w, Ws_coff, Wc_coff):
    from concourse.bass_utils import run_bass_kernel_spmd

    h_context = np.asarray(h_context, np.float32)
    h_structure = np.asarray(h_structure, np.float32)
    Wc_w = np.asarray(Wc_w, np.float32)
    Wc_b = np.asarray(Wc_b, np.float32)
    Ws_w = np.asarray(Ws_w, np.float32)
    Ws_b = np.asarray(Ws_b, np.float32)
    ac_w = np.asarray(ac_w, np.float32)
    as_w = np.asarray(as_w, np.float32)
    ei = np.asarray(edge_index)

    wA = float(abs(np.float32(np.asarray(Ws_coff)[0, 0])))  # scales alpha_c
    wB = float(abs(np.float32(np.asarray(Wc_coff)[0, 0])))  # scales alpha_s

    vA_np = np.stack([Wc_w.T @ ac_w[0, :F], Wc_w.T @ ac_w[0, F:]], axis=1) * wA
    uB_np = np.stack([
        np.ones(K, np.float32),
        wB * (Ws_w.T @ as_w[0, F:]),   # dstB proj
        wB * (Ws_w.T @ as_w[0, :F]),   # srcB proj
    ], axis=1).astype(np.float32)
    cA = wA * float(Wc_b @ ac_w[0, :F] + Wc_b @ ac_w[0, F:])
    cB = wB * float(Ws_b @ as_w[0, :F] + Ws_b @ as_w[0, F:])

    key = (round(cA, 12), round(cB, 12))
    if key not in _BUILD_CACHE:
        _BUILD_CACHE[key] = _build_program(cA, cB)
    nc = _BUILD_CACHE[key]

    # edge-indicator adjacency, transposed + partition-major re-layout
    maskCT = np.zeros((N, N), np.uint8)
    maskCT[ei[1], ei[0]] = 1

    hctxT = np.ascontiguousarray(h_context.T.astype(np.float16))
    hstrT = np.ascontiguousarray(h_structure.T.astype(np.float16))
    vA_f16 = np.ascontiguousarray(vA_np.astype(np.float16))
    wvA_f16 = np.ascontiguousarray(
        np.concatenate([Wc_w.T, vA_np[:, 1:2]], axis=1).astype(np.float16))
    uB_f16 = np.ascontiguousarray(uB_np.astype(np.float16))

    # host M0 shim: upper bound of alpha per core (cancels in division)
    lrelu = lambda x: np.where(x > 0, x, 0.01 * x)
    srcA = h_context @ (vA_np[:, 0]) + cA          # wA folded
    dstA = h_context @ (vA_np[:, 1])
    e_str = np.exp(h_structure - h_structure.max(axis=1, keepdims=True))
    sm = e_str / e_str.sum(axis=1, keepdims=True)
    srcB = sm @ uB_np[:, 2] + cB
    dstB = sm @ uB_np[:, 1]
    dstA_max = float(dstA.max())
    dstB_max = float(dstB.max())

    in_maps = []
    for d in range(NC):
        sl = slice(S * d, S * (d + 1))
        m0_d = (lrelu(float(srcA[sl].max()) + dstA_max)
                + lrelu(float(srcB[sl].max()) + dstB_max))
        maskP = np.ascontiguousarray(
            maskCT[:, sl].reshape(N // 128, 128, S)
            .transpose(1, 0, 2).reshape(128, (N // 128) * S))
        in_maps.append({
            "hctxT": hctxT,
            "hstrT": hstrT,
            "hctxT_my": np.ascontiguousarray(hctxT[:, sl]),
            "hstrT_my": np.ascontiguousarray(hstrT[:, sl]),
            "wvA": wvA_f16,
            "vA": vA_f16,
            "uB": uB_f16,
            "maskP": maskP,
            "negm0": np.full((128, 1), -np.float32(m0_d), np.float32),
        })

    res = run_bass_kernel_spmd(nc, in_maps, core_ids=list(range(NC)))
    out = np.empty((N, F), np.float32)
    for d in range(NC):
        out[S * d:S * (d + 1), :] = res.results[d]["outT"].T

    # rows with no edges: reference gives uniform attention = mean of hc
    row_deg = np.zeros(N, np.int64)
    np.add.at(row_deg, ei[0], 1)
    empty = row_deg == 0
    if empty.any():
        hc_host = h_context @ Wc_w.T + Wc_b
        out[empty, :] = hc_host.mean(axis=0)

    return out
